# revision 2
# baseline (speedup 1.0000x reference)
"""Trainium2 kernel for out = A @ W2 @ B.T with banded Gaussian W2.

Math: W2 = W1*W1 where W1[i,j] = exp(-(i-j)^2/(2*8^2)) truncated below 1e-10.
W1 > eps only for |i-j| <= 54, so in 128-blocks W2 is block-tridiagonal AND
translation-invariant: only three distinct 128x128 blocks exist (diag D0,
super-diag U = W2[j-1,j], sub-diag L = W2[j+1,j] = U.T).

Strategy (data-parallel over A's rows, 8 cores, no collectives):
  - host: transpose A and B once, cast to bf16, build the three W2 blocks.
  - each core gets A.T slab [4096, 1024] bf16, full B.T bf16, the W2 pack.
  - phase 1 (once): TT = W2 @ A.T  (= (A_slab @ W2).T), banded block-tridiag
    matmuls over the narrow A-slab; TT [4096, 1024] bf16 stays in SBUF.
  - phase 2 (per 512-col chunk nu): out[:, nu] = TT.T @ B.T[:, nu], with all
    8 PSUM banks accumulating the 8 m-tiles while B.T streams through once.
  - bf16 operands: 1 cyc/row on the PE like fp32r, but LDWEIGHTS gets the
    fast-weight-load path (disabled for fp32 dtypes) and DMA bytes halve.
    Accumulation stays fp32 in PSUM; output is written fp32.
"""

import numpy as np
import ml_dtypes

import concourse.bass as bass
import concourse.mybir as mybir
from concourse import bacc
from concourse.bass_utils import run_bass_kernel_spmd
from concourse.tile import TileContext

P = 128          # partition / block size
N = 4096         # inner dims (A cols, B rows/cols)
M_FULL = 8192    # A rows
NCORES = 8
MS = M_FULL // NCORES   # 1024 rows of A per core
NK = N // P      # 32 contraction blocks
NM = MS // P     # 8 m-tiles per core
CW = 512         # output column chunk width (= 1 PSUM bank of fp32)
NCH = N // CW    # 8 chunks
NH = MS // CW    # 2 column-halves of the A.T slab in phase 1

SIGMA = 8.0
TRUNC_EPS = 1e-10
BF16 = ml_dtypes.bfloat16

_COMPILED = {}


def _w2_block(dist):
    """W2 entries for a matrix of absolute diagonal distances."""
    d = dist.astype(np.float32)
    w1 = np.exp(-(d * d) / np.float32(2.0 * SIGMA * SIGMA)).astype(np.float32)
    w1 = np.where(w1 > np.float32(TRUNC_EPS), w1, np.float32(0.0)).astype(np.float32)
    return (w1 * w1).astype(np.float32)


def _build_w2_pack():
    a = np.arange(P)[:, None]
    b = np.arange(P)[None, :]
    d0 = _w2_block(np.abs(a - b))          # W2[j, j]
    u = _w2_block(np.abs(a - b - P))       # W2[j-1, j]
    l = _w2_block(np.abs(P + a - b))       # W2[j+1, j]
    pack = np.concatenate([d0, u, l], axis=1)  # [128, 384]
    return np.ascontiguousarray(pack.astype(BF16))


def _prep_inputs(A, B):
    """Host-side prep: transpose + bf16-cast, shard A.T by core."""
    a_t = np.ascontiguousarray(np.asarray(A, dtype=np.float32).T.astype(BF16))
    b_t = np.ascontiguousarray(np.asarray(B, dtype=np.float32).T.astype(BF16))
    w2_pack = _build_w2_pack()
    return [
        {
            "at": np.ascontiguousarray(a_t[:, c * MS:(c + 1) * MS]),
            "bt": b_t,
            "w2": w2_pack,
        }
        for c in range(NCORES)
    ]


def _build_program(reps=1):
    """Build + compile the Bass program (one NEFF, run SPMD on 8 cores)."""
    nc = bacc.Bacc("TRN2", target_bir_lowering=False, debug=False)
    f32 = mybir.dt.float32
    bf16 = mybir.dt.bfloat16

    at_dram = nc.dram_tensor("at", [N, MS], bf16, kind="ExternalInput").ap()
    bt_dram = nc.dram_tensor("bt", [N, N], bf16, kind="ExternalInput").ap()
    w2_dram = nc.dram_tensor("w2", [P, 3 * P], bf16, kind="ExternalInput").ap()
    out_dram = nc.dram_tensor("out", [MS, N], f32, kind="ExternalOutput").ap()

    with TileContext(nc) as tc:
        with (
            tc.tile_pool(name="const", bufs=1) as const_pool,
            tc.tile_pool(name="atp", bufs=6) as at_pool,
            tc.tile_pool(name="ttp", bufs=1) as tt_pool,
            tc.tile_pool(name="btp", bufs=4) as bt_pool,
            tc.tile_pool(name="obp", bufs=4) as ob_pool,
            tc.tile_pool(name="psp", bufs=8, space="PSUM") as ps_pool,
        ):
            w2_sb = const_pool.tile([P, 3 * P], bf16, tag="w2", name="w2_sb")
            nc.sync.dma_start(w2_sb, w2_dram)
            # lhsT for contribution d: W2[j+d, j]
            w2_lhsT = {
                0: w2_sb[:, 0:P],
                -1: w2_sb[:, P:2 * P],
                1: w2_sb[:, 2 * P:3 * P],
            }

            for rep in range(reps):
                # --- phase 1: TT = W2 @ A.T ([4096, 1024], resident in SBUF)
                at_tiles = [None] * NK

                def get_at(k, rep=rep):
                    if at_tiles[k] is None:
                        at_t = at_pool.tile([P, MS], bf16, tag="at",
                                            name=f"at_sb_{rep}_{k}")
                        nc.sync.dma_start(at_t, at_dram[k * P:(k + 1) * P, :])
                        at_tiles[k] = at_t
                    return at_tiles[k]

                tt_tiles = []
                for j in range(NK):
                    tt_t = tt_pool.tile([P, MS], bf16, tag=f"tt{j}",
                                        name=f"tt_sb_{rep}_{j}")
                    dlist = [d for d in (-1, 0, 1) if 0 <= j + d < NK]
                    ps_t = [
                        ps_pool.tile([P, CW], f32, tag="ps",
                                     name=f"ps_t_{rep}_{j}_{h}")
                        for h in range(NH)
                    ]
                    # d outer / h inner: each W2 LDWEIGHTS serves NH matmuls
                    for i, d in enumerate(dlist):
                        for h in range(NH):
                            nc.tensor.matmul(
                                ps_t[h],
                                lhsT=w2_lhsT[d],
                                rhs=get_at(j + d)[:, bass.ts(h, CW)],
                                start=(i == 0),
                                stop=(i == len(dlist) - 1),
                            )
                    for h in range(NH):
                        nc.vector.tensor_copy(tt_t[:, bass.ts(h, CW)], ps_t[h])
                    tt_tiles.append(tt_t)

                # --- phase 2: out = TT.T @ B.T, streamed in 512-col chunks
                for nu in range(NCH):
                    cs = bass.ts(nu, CW)
                    ps_o = [
                        ps_pool.tile([P, CW], f32, tag="ps",
                                     name=f"ps_o_{rep}_{nu}_{m}")
                        for m in range(NM)
                    ]
                    for k in range(NK):
                        bt_t = bt_pool.tile([P, CW], bf16, tag="bt",
                                            name=f"bt_sb_{rep}_{nu}_{k}")
                        nc.sync.dma_start(
                            bt_t, bt_dram[k * P:(k + 1) * P, cs]
                        )
                        for m in range(NM):
                            nc.tensor.matmul(
                                ps_o[m],
                                lhsT=tt_tiles[k][:, m * P:(m + 1) * P],
                                rhs=bt_t,
                                start=(k == 0),
                                stop=(k == NK - 1),
                            )
                    for m in range(NM):
                        ob_t = ob_pool.tile([P, CW], f32, tag="ob",
                                            name=f"ob_sb_{rep}_{nu}_{m}")
                        if m % 2 == 0:
                            nc.vector.tensor_copy(ob_t, ps_o[m])
                        else:
                            nc.scalar.copy(ob_t, ps_o[m])
                        nc.sync.dma_start(
                            out_dram[m * P:(m + 1) * P, cs], ob_t
                        )

    nc.compile()
    return nc


def _get_program():
    if "nc" not in _COMPILED:
        _COMPILED["nc"] = _build_program()
    return _COMPILED["nc"]


def kernel(A, B):
    A = np.asarray(A, dtype=np.float32)
    B = np.asarray(B, dtype=np.float32)
    assert A.shape == (M_FULL, N), A.shape
    assert B.shape == (N, N), B.shape

    in_maps = _prep_inputs(A, B)
    nc = _get_program()
    res = run_bass_kernel_spmd(nc, in_maps, core_ids=list(range(NCORES)))
    return np.concatenate(
        [res.results[c]["out"] for c in range(NCORES)], axis=0
    ).astype(np.float32)


# revision 4
# speedup vs baseline: 5.6108x; 5.6108x over previous
"""Trainium2 kernel for out = A @ W2 @ B.T with banded Gaussian W2.

Math: W2 = W1*W1 where W1[i,j] = exp(-(i-j)^2/(2*8^2)) truncated below 1e-10.
W1 > eps only for |i-j| <= 54, so in 128-blocks W2 is block-tridiagonal AND
translation-invariant: only three distinct 128x128 blocks exist (diag D0,
super-diag U = W2[j-1,j], sub-diag L = W2[j+1,j] = U.T).

Strategy (data-parallel over A's rows, 8 cores, no collectives):
  - host: transpose A and B once, cast to bf16, build the three W2 blocks.
  - each core gets A.T slab [4096, 1024] bf16, full B.T bf16, the W2 pack.
  - phase 1 (once): TT = W2 @ A.T  (= (A_slab @ W2).T), banded block-tridiag
    matmuls over the narrow A-slab; TT [4096, 1024] bf16 stays in SBUF.
  - phase 2 (per 512-col chunk nu): out[:, nu] = TT.T @ B.T[:, nu], with all
    8 PSUM banks accumulating the 8 m-tiles while B.T streams through once.
  - bf16 operands: 1 cyc/row on the PE like fp32r, but LDWEIGHTS gets the
    fast-weight-load path (disabled for fp32 dtypes) and DMA bytes halve.
    Accumulation stays fp32 in PSUM; output is stored bf16 and widened to
    fp32 on the host.
  - DMA batching: B.T streams as one DMA per 4 contraction blocks (strided
    AP), output as one DMA per 512-col chunk — 105 DMA instructions total
    instead of 353, relieving the HWDGE/sequencer issue path.
"""

import numpy as np
import ml_dtypes

import concourse.bass as bass
import concourse.mybir as mybir
from concourse import bacc
from concourse.bass_utils import run_bass_kernel_spmd
from concourse.tile import TileContext

P = 128          # partition / block size
N = 4096         # inner dims (A cols, B rows/cols)
M_FULL = 8192    # A rows
NCORES = 8
MS = M_FULL // NCORES   # 1024 rows of A per core
NK = N // P      # 32 contraction blocks
NM = MS // P     # 8 m-tiles per core
CW = 512         # output column chunk width (= 1 PSUM bank of fp32)
NCH = N // CW    # 8 chunks
NH = MS // CW    # 2 column-halves of the A.T slab in phase 1
KB = 4           # contraction blocks per bt DMA batch
NKG = NK // KB   # 8 bt DMA batches per chunk

SIGMA = 8.0
TRUNC_EPS = 1e-10
BF16 = ml_dtypes.bfloat16

_COMPILED = {}


def _w2_block(dist):
    """W2 entries for a matrix of absolute diagonal distances."""
    d = dist.astype(np.float32)
    w1 = np.exp(-(d * d) / np.float32(2.0 * SIGMA * SIGMA)).astype(np.float32)
    w1 = np.where(w1 > np.float32(TRUNC_EPS), w1, np.float32(0.0)).astype(np.float32)
    return (w1 * w1).astype(np.float32)


def _build_w2_pack():
    a = np.arange(P)[:, None]
    b = np.arange(P)[None, :]
    d0 = _w2_block(np.abs(a - b))          # W2[j, j]
    u = _w2_block(np.abs(a - b - P))       # W2[j-1, j]
    l = _w2_block(np.abs(P + a - b))       # W2[j+1, j]
    pack = np.concatenate([d0, u, l], axis=1)  # [128, 384]
    return np.ascontiguousarray(pack.astype(BF16))


def _prep_inputs(A, B):
    """Host-side prep: transpose + bf16-cast, shard A.T by core."""
    a_t = np.ascontiguousarray(np.asarray(A, dtype=np.float32).T.astype(BF16))
    b_t = np.ascontiguousarray(np.asarray(B, dtype=np.float32).T.astype(BF16))
    w2_pack = _build_w2_pack()
    return [
        {
            "at": np.ascontiguousarray(a_t[:, c * MS:(c + 1) * MS]),
            "bt": b_t,
            "w2": w2_pack,
        }
        for c in range(NCORES)
    ]


def _build_program(reps=1):
    """Build + compile the Bass program (one NEFF, run SPMD on 8 cores)."""
    nc = bacc.Bacc("TRN2", target_bir_lowering=False, debug=False)
    f32 = mybir.dt.float32
    bf16 = mybir.dt.bfloat16

    at_dram = nc.dram_tensor("at", [N, MS], bf16, kind="ExternalInput").ap()
    bt_dram = nc.dram_tensor("bt", [N, N], bf16, kind="ExternalInput").ap()
    w2_dram = nc.dram_tensor("w2", [P, 3 * P], bf16, kind="ExternalInput").ap()
    out_dram = nc.dram_tensor("out", [MS, N], bf16, kind="ExternalOutput").ap()
    # [m-tile, row-in-tile, col] view for the batched per-chunk store
    out_g = out_dram.rearrange("(m p) c -> p m c", p=P)

    with TileContext(nc) as tc:
        with (
            tc.tile_pool(name="const", bufs=1) as const_pool,
            tc.tile_pool(name="atp", bufs=6) as at_pool,
            tc.tile_pool(name="ttp", bufs=1) as tt_pool,
            tc.tile_pool(name="btp", bufs=3) as bt_pool,
            tc.tile_pool(name="obp", bufs=2) as ob_pool,
            tc.tile_pool(name="psp", bufs=8, space="PSUM") as ps_pool,
        ):
            w2_sb = const_pool.tile([P, 3 * P], bf16, tag="w2", name="w2_sb")
            nc.sync.dma_start(w2_sb, w2_dram)
            # lhsT for contribution d: W2[j+d, j]
            w2_lhsT = {
                0: w2_sb[:, 0:P],
                -1: w2_sb[:, P:2 * P],
                1: w2_sb[:, 2 * P:3 * P],
            }

            for rep in range(reps):
                # --- phase 1: TT = W2 @ A.T ([4096, 1024], resident in SBUF)
                at_tiles = [None] * NK

                def get_at(k, rep=rep):
                    if at_tiles[k] is None:
                        at_t = at_pool.tile([P, MS], bf16, tag="at",
                                            name=f"at_sb_{rep}_{k}")
                        nc.sync.dma_start(at_t, at_dram[k * P:(k + 1) * P, :])
                        at_tiles[k] = at_t
                    return at_tiles[k]

                tt_tiles = []
                for j in range(NK):
                    tt_t = tt_pool.tile([P, MS], bf16, tag=f"tt{j}",
                                        name=f"tt_sb_{rep}_{j}")
                    dlist = [d for d in (-1, 0, 1) if 0 <= j + d < NK]
                    ps_t = [
                        ps_pool.tile([P, CW], f32, tag="ps",
                                     name=f"ps_t_{rep}_{j}_{h}")
                        for h in range(NH)
                    ]
                    for i, d in enumerate(dlist):
                        for h in range(NH):
                            nc.tensor.matmul(
                                ps_t[h],
                                lhsT=w2_lhsT[d],
                                rhs=get_at(j + d)[:, bass.ts(h, CW)],
                                start=(i == 0),
                                stop=(i == len(dlist) - 1),
                            )
                    for h in range(NH):
                        nc.vector.tensor_copy(tt_t[:, bass.ts(h, CW)], ps_t[h])
                    tt_tiles.append(tt_t)

                # --- phase 2: out = TT.T @ B.T, streamed in 512-col chunks
                for nu in range(NCH):
                    cs = bass.ts(nu, CW)
                    ps_o = [
                        ps_pool.tile([P, CW], f32, tag="ps",
                                     name=f"ps_o_{rep}_{nu}_{m}")
                        for m in range(NM)
                    ]
                    ob_t = ob_pool.tile([P, NM * CW], bf16, tag="ob",
                                        name=f"ob_sb_{rep}_{nu}")
                    for kg in range(NKG):
                        # one DMA per KB k-blocks: [p, kb, c] strided view
                        bt_t = bt_pool.tile([P, KB * CW], bf16, tag="bt",
                                            name=f"bt_sb_{rep}_{nu}_{kg}")
                        src = bt_dram[kg * KB * P:(kg + 1) * KB * P, cs]
                        nc.sync.dma_start(
                            bt_t, src.rearrange("(g p) c -> p g c", p=P)
                        )
                        for g in range(KB):
                            k = kg * KB + g
                            rhs = bt_t[:, g * CW:(g + 1) * CW]
                            for m in range(NM):
                                nc.tensor.matmul(
                                    ps_o[m],
                                    lhsT=tt_tiles[k][:, m * P:(m + 1) * P],
                                    rhs=rhs,
                                    start=(k == 0),
                                    stop=(k == NK - 1),
                                )
                    for m in range(NM):
                        dst = ob_t[:, m * CW:(m + 1) * CW]
                        if m % 2 == 0:
                            nc.vector.tensor_copy(dst, ps_o[m])
                        else:
                            nc.scalar.copy(dst, ps_o[m])
                    nc.sync.dma_start(
                        out_g[:, :, nu * CW:(nu + 1) * CW], ob_t
                    )

    nc.compile()
    return nc


def _get_program():
    if "nc" not in _COMPILED:
        _COMPILED["nc"] = _build_program()
    return _COMPILED["nc"]


def kernel(A, B):
    A = np.asarray(A, dtype=np.float32)
    B = np.asarray(B, dtype=np.float32)
    assert A.shape == (M_FULL, N), A.shape
    assert B.shape == (N, N), B.shape

    in_maps = _prep_inputs(A, B)
    nc = _get_program()
    res = run_bass_kernel_spmd(nc, in_maps, core_ids=list(range(NCORES)))
    return np.concatenate(
        [np.asarray(res.results[c]["out"]) for c in range(NCORES)], axis=0
    ).astype(np.float32)
